# revision 8
# baseline (speedup 1.0000x reference)
# Trainium2 Bass kernel for nn_LAB_42906723287350.
#
#   probs = softmax(choice_parameters, axis=0); s = x @ probs
#   out = mix0*multilinear(sigmoid(lut); s) + mix1*clip(s0+s1+s2-2,0,1)
#         + mix2*(s0+s1+s2>=2)
#
# Data parallel over 8 cores (R=131072 rows each).  Row mapping per core:
#   row = ROWS_TILE*nd + T_IL*p + t   (nd = tile, p = partition, t in [0,T_IL))
# Per tile nd: SWDGE cast-DMA x [128, (t,c)] f32->bf16 (4KB/partition reads)
#   -> T_IL/2 PE transposes (bf16) -> xT [(t2,c), p] psum bf16 -> copy
#   -> T_IL/2 accumulating matmuls with xT as STATIONARY and P [128, 4*T_IL]
#      moving -> s in NATURAL layout [128 p, (j,t)] (no back-transpose).
# Per group g (TILES_GRP tiles): copy s psum->SBUF, 16-coeff Horner poly +
#   add path -> vo2 [128, (t,k)] -> PE transposes -> [(tl,k), p] psum
#   -> strided copies into S [(g,k), (p,t)] -> contiguous output DMA
#   (T_IL*512B/partition descriptors).
import numpy as np
import ml_dtypes

import concourse.bass as bass
import concourse.mybir as mybir
import concourse.tile as tile
from concourse import bacc
from concourse.bass_utils import run_bass_kernel_spmd
from concourse.masks import make_identity

N_CORES = 8
B_FULL = 1048576
CIN = 64
F32 = mybir.dt.float32
MM_DT = mybir.dt.bfloat16
ALU = mybir.AluOpType
AF = mybir.ActivationFunctionType

T_IL = 16                        # rows interleaved per partition
H_BLK = T_IL // 2                # 128-partition transpose sub-blocks per tile
ROWS_TILE = 128 * T_IL           # 2048 rows per x-tile
TILES_GRP = 32                   # tiles per poly group
ROWS_GRP = ROWS_TILE * TILES_GRP # 65536 rows
JT = 4 * T_IL                    # s-matmul output cols per tile (j,t)
TPB = 512 // JT                  # tiles per PSUM bank for s accumulation
TB = 128 // TILES_GRP            # t values per vo transpose block


def build_nc(R, mm_dtype=MM_DT):
    n_tiles = R // ROWS_TILE
    n_grp = n_tiles // TILES_GRP
    assert R == n_grp * ROWS_GRP and n_grp * TILES_GRP <= 128

    nc = bacc.Bacc()
    x_d = nc.dram_tensor("x", [R, CIN], F32, kind="ExternalInput")
    pm_d = nc.dram_tensor("pm", [128, H_BLK * JT], mm_dtype, kind="ExternalInput")
    coef_d = nc.dram_tensor("coef", [128, 18], F32, kind="ExternalInput")
    out_d = nc.dram_tensor("out", [R, 1], F32, kind="ExternalOutput")

    # x row = ROWS_TILE*nd + T_IL*p + t ; tile view [nd, p, (t c)]
    x2v = x_d[:].rearrange("(nd p t) c -> nd p (t c)", p=128, t=T_IL)
    # out row = ROWS_TILE*(g*TILES_GRP+k) + T_IL*p + t ; [(g k), (p t)]
    outv = out_d[:].rearrange(
        "(gk p t) one -> gk (p t one)", gk=n_grp * TILES_GRP, p=128, t=T_IL
    )

    with tile.TileContext(nc) as tc:
        with (
            tc.tile_pool(name="const", bufs=1) as cpool,
            tc.tile_pool(name="xin", bufs=10) as xpool,
            tc.tile_pool(name="xtsb", bufs=8) as xsbpool,
            tc.tile_pool(name="tmp", bufs=2) as tpool,
            tc.tile_pool(name="outsb", bufs=1) as spool,
            tc.tile_pool(name="psxt", bufs=3, space="PSUM") as ppxt,
            tc.tile_pool(name="pssn", bufs=2, space="PSUM") as ppsn,
            tc.tile_pool(name="psvo", bufs=1, space="PSUM") as ppvo,
        ):
            identb = cpool.tile([128, 128], mm_dtype)
            make_identity(nc, identb[:])
            identf = cpool.tile([128, 128], F32)
            make_identity(nc, identf[:])
            pm_sb = cpool.tile([128, H_BLK * JT], mm_dtype)
            nc.sync.dma_start(out=pm_sb[:], in_=pm_d[:])
            coef_sb = cpool.tile([128, 18], F32)
            nc.sync.dma_start(out=coef_sb[:], in_=coef_d[:])

            S = spool.tile([n_grp * TILES_GRP, 128 * T_IL], F32)  # [(g k), (p t)]
            Sv = S[:].rearrange("q (p t) -> q p t", p=128, t=T_IL)

            KH = TILES_GRP // 2   # tiles per poly half

            def poly_half(sn_ps, vo2, half):
                # sn_ps [128, (k j t)] f32 (PSUM); s_j(row) at [p, k, j, t]
                sv = sn_ps[:].rearrange(
                    "p (k j t) -> p k j t", k=KH, j=4, t=T_IL
                )
                s = [sv[:, :, jj, :] for jj in range(4)]

                def tmp(tag):
                    tl = tpool.tile([128, KH * T_IL], F32, tag=tag)
                    return tl, tl[:].rearrange(
                        "p (k t) -> p k t", k=KH, t=T_IL
                    )

                Lf = []
                for i in range(8):
                    _, v = tmp(f"leaf{i}")
                    nc.scalar.activation(
                        v, s[0], AF.Identity,
                        bias=coef_sb[:, i : i + 1],
                        scale=coef_sb[:, 8 + i : 9 + i],
                    )
                    Lf.append(v)
                G = []
                for k in range(4):
                    _, v = tmp(f"gm{k}")
                    nc.vector.tensor_mul(v, s[1], Lf[2 * k + 1])
                    nc.vector.tensor_add(v, v, Lf[2 * k])
                    G.append(v)
                H = []
                for m in range(2):
                    _, v = tmp(f"hm{m}")
                    nc.vector.tensor_mul(v, s[2], G[2 * m + 1])
                    nc.vector.tensor_add(v, v, G[2 * m])
                    H.append(v)
                _, vl = tmp("lut")
                nc.vector.tensor_mul(vl, s[3], H[1])
                nc.vector.tensor_add(vl, vl, H[0])
                _, va0 = tmp("adds0")
                nc.vector.tensor_copy(out=va0, in_=s[0])
                _, va = tmp("adds")
                nc.vector.tensor_add(va, va0, s[1])
                nc.vector.tensor_add(va, va, s[2])
                _, vr = tmp("relu")
                nc.vector.tensor_scalar(vr, va, -2.0, 0.0, op0=ALU.add, op1=ALU.max)
                nc.vector.tensor_scalar(
                    vr, vr, 1.0, coef_sb[:, 16:17], op0=ALU.min, op1=ALU.mult
                )
                _, vq = tmp("step")
                nc.vector.tensor_scalar(
                    vq, va, 2.0, coef_sb[:, 17:18], op0=ALU.is_ge, op1=ALU.mult
                )
                # write into this half's k-slice of the group vo2 [128, (t k)]
                vo2v = vo2[:].rearrange("p (t k) -> p k t", t=T_IL, k=TILES_GRP)[
                    :, KH * half : KH * (half + 1), :
                ]
                nc.vector.tensor_add(vo2v, vl, vr)
                nc.vector.tensor_add(vo2v, vo2v, vq)

            def store_group(vo2, g):
                # transposes: block b -> voT[:, 128b:...] = [(tl k), p], t = TB*b+tl
                n_blk = TILES_GRP * T_IL // 128
                voT = ppvo.tile([128, 128 * n_blk], F32, tag="voT")
                for b in range(n_blk):
                    nc.tensor.transpose(
                        voT[:, 128 * b : 128 * (b + 1)],
                        vo2[:, 128 * b : 128 * (b + 1)],
                        identf[:],
                    )
                for t in range(T_IL):
                    b, tl = divmod(t, TB)
                    dst = Sv[TILES_GRP * g : TILES_GRP * (g + 1), :, t]
                    srcv = voT[
                        TILES_GRP * tl : TILES_GRP * (tl + 1),
                        128 * b : 128 * (b + 1),
                    ]
                    if t % 2 == 0:
                        nc.scalar.copy(out=dst, in_=srcv)
                    else:
                        nc.vector.tensor_copy(out=dst, in_=srcv)
                nc.sync.dma_start(
                    out=outv[TILES_GRP * g : TILES_GRP * (g + 1)],
                    in_=S[TILES_GRP * g : TILES_GRP * (g + 1), :],
                )

            for g in range(n_grp):
                vo2 = tpool.tile([128, TILES_GRP * T_IL], F32, tag="vo2")
                for half in range(2):
                    sn_ps = ppsn.tile([128, JT * (TILES_GRP // 2)], F32, tag="sn")
                    for kk in range(TILES_GRP // 2):
                        k = (TILES_GRP // 2) * half + kk
                        nd = g * TILES_GRP + k
                        xt = xpool.tile([128, 128 * H_BLK], mm_dtype, tag="x")
                        nc.gpsimd.dma_start(out=xt[:], in_=x2v[nd])
                        xT_ps = ppxt.tile([128, 128 * H_BLK], mm_dtype, tag="xT")
                        for h in range(H_BLK):
                            nc.tensor.transpose(
                                xT_ps[:, 128 * h : 128 * (h + 1)],
                                xt[:, 128 * h : 128 * (h + 1)],
                                identb[:],
                            )
                        xT_sb = xsbpool.tile([128, 128 * H_BLK], mm_dtype, tag="xTsb")
                        if nd % 2 == 0:
                            nc.scalar.copy(out=xT_sb[:], in_=xT_ps[:])
                        else:
                            nc.vector.tensor_copy(out=xT_sb[:], in_=xT_ps[:])
                        for h in range(H_BLK):
                            nc.tensor.matmul(
                                sn_ps[:, JT * kk : JT * (kk + 1)],
                                lhsT=xT_sb[:, 128 * h : 128 * (h + 1)],
                                rhs=pm_sb[:, JT * h : JT * (h + 1)],
                                start=(h == 0),
                                stop=(h == H_BLK - 1),
                            )
                    poly_half(sn_ps, vo2, half)
                store_group(vo2, g)
    nc.compile()
    return nc


def host_prep(choice_parameters, lut, lut_vs_add_choice_parameters, mm_np=ml_dtypes.bfloat16):
    cp = np.asarray(choice_parameters, dtype=np.float64)
    e = np.exp(cp - cp.max(axis=0, keepdims=True))
    probs = e / e.sum(axis=0, keepdims=True)  # [64,4]
    L = 1.0 / (1.0 + np.exp(-np.asarray(lut, dtype=np.float64)))
    m = np.asarray(lut_vs_add_choice_parameters, dtype=np.float64)
    em = np.exp(m - m.max())
    mix = em / em.sum()

    c = np.zeros(16)
    for Sm in range(16):
        v = L
        for ax in range(4):
            vec = np.array([1.0, -1.0]) if (Sm >> ax) & 1 else np.array([0.0, 1.0])
            v = np.tensordot(v, vec, axes=([0], [0]))
        c[Sm] = float(v) * mix[0]

    coef_row = np.zeros(18)
    for idx in range(8):
        coef_row[idx] = c[idx << 1]
        coef_row[8 + idx] = c[(idx << 1) | 1]
    coef_row[16] = mix[1]
    coef_row[17] = mix[2]
    coef = np.tile(coef_row.astype(np.float32)[None], (128, 1))

    # pm[t2*64+c, JT*h + T_IL*j + t] = probs[c,j] * [t == 2h+t2]
    pm = np.zeros((128, H_BLK * JT), np.float64)
    for h in range(H_BLK):
        for t2 in range(2):
            for cc in range(64):
                for j in range(4):
                    pm[t2 * 64 + cc, JT * h + T_IL * j + (2 * h + t2)] = probs[cc, j]
    pm = pm.astype(mm_np)
    return pm, coef


_NC_CACHE = {}


def _get_nc(R):
    if R not in _NC_CACHE:
        _NC_CACHE[R] = build_nc(R)
    return _NC_CACHE[R]


def run_on_hw(x, choice_parameters, lut, lut_vs_add_choice_parameters, **kw):
    x = np.ascontiguousarray(np.asarray(x, dtype=np.float32))
    R = x.shape[0] // N_CORES
    nc = _get_nc(R)
    pm, coef = host_prep(choice_parameters, lut, lut_vs_add_choice_parameters)
    in_maps = [
        {"x": np.ascontiguousarray(x[i * R : (i + 1) * R]), "pm": pm, "coef": coef}
        for i in range(N_CORES)
    ]
    res = run_bass_kernel_spmd(nc, in_maps, list(range(N_CORES)), **kw)
    out = np.concatenate([r["out"] for r in res.results], axis=0)
    return out, res


def kernel(x, choice_parameters, lut, lut_vs_add_choice_parameters):
    out, _ = run_on_hw(x, choice_parameters, lut, lut_vs_add_choice_parameters)
    return out


# revision 9
# speedup vs baseline: 1.1125x; 1.1125x over previous
# Trainium2 Bass kernel for nn_LAB_42906723287350.
#
#   probs = softmax(choice_parameters, axis=0); s = x @ probs
#   out = mix0*multilinear(sigmoid(lut); s) + mix1*clip(s0+s1+s2-2,0,1)
#         + mix2*(s0+s1+s2>=2)
#
# Data parallel over 8 cores (R=131072 rows each).  Row mapping per core:
#   row = ROWS_TILE*nd + T_IL*p + t   (nd = tile, p = partition, t in [0,T_IL))
# Per tile nd: SWDGE cast-DMA x [128, (t,c)] f32->bf16 (4KB/partition reads)
#   -> T_IL/2 PE transposes (bf16) -> xT [(t2,c), p] psum bf16 -> copy
#   -> T_IL/2 accumulating matmuls with xT as STATIONARY and P [128, 4*T_IL]
#      moving -> s in NATURAL layout [128 p, (j,t)] (no back-transpose).
# Per group g (TILES_GRP tiles): copy s psum->SBUF, 16-coeff Horner poly +
#   add path -> vo2 [128, (t,k)] -> PE transposes -> [(tl,k), p] psum
#   -> strided copies into S [(g,k), (p,t)] -> contiguous output DMA
#   (T_IL*512B/partition descriptors).
import numpy as np
import ml_dtypes

import concourse.bass as bass
import concourse.mybir as mybir
import concourse.tile as tile
from concourse import bacc
from concourse.bass_utils import run_bass_kernel_spmd
from concourse.masks import make_identity

N_CORES = 8
B_FULL = 1048576
CIN = 64
F32 = mybir.dt.float32
MM_DT = mybir.dt.bfloat16
ALU = mybir.AluOpType
AF = mybir.ActivationFunctionType

T_IL = 16                        # rows interleaved per partition
H_BLK = T_IL // 2                # 128-partition transpose sub-blocks per tile
ROWS_TILE = 128 * T_IL           # 2048 rows per x-tile
TILES_GRP = 32                   # tiles per poly group
ROWS_GRP = ROWS_TILE * TILES_GRP # 65536 rows
JT = 4 * T_IL                    # s-matmul output cols per tile (j,t)
TPB = 512 // JT                  # tiles per PSUM bank for s accumulation
TB = 128 // TILES_GRP            # t values per vo transpose block


def build_nc(R, mm_dtype=MM_DT):
    n_tiles = R // ROWS_TILE
    n_grp = n_tiles // TILES_GRP
    assert R == n_grp * ROWS_GRP and n_grp * TILES_GRP <= 128

    nc = bacc.Bacc()
    x_d = nc.dram_tensor("x", [R, CIN], F32, kind="ExternalInput")
    pm_d = nc.dram_tensor("pm", [128, H_BLK * JT], mm_dtype, kind="ExternalInput")
    coef_d = nc.dram_tensor("coef", [128, 18], F32, kind="ExternalInput")
    out_d = nc.dram_tensor("out", [R, 1], F32, kind="ExternalOutput")

    # x row = ROWS_TILE*nd + T_IL*p + t ; tile view [nd, p, (t c)]
    x2v = x_d[:].rearrange("(nd p t) c -> nd p (t c)", p=128, t=T_IL)
    # out row = ROWS_TILE*(g*TILES_GRP+k) + T_IL*p + t ; [(g k), (p t)]
    outv = out_d[:].rearrange(
        "(gk p t) one -> gk (p t one)", gk=n_grp * TILES_GRP, p=128, t=T_IL
    )

    with tile.TileContext(nc) as tc:
        with (
            tc.tile_pool(name="const", bufs=1) as cpool,
            tc.tile_pool(name="xin", bufs=8) as xpool,
            tc.tile_pool(name="xtsb", bufs=6) as xsbpool,
            tc.tile_pool(name="tmp", bufs=2) as tpool,
            tc.tile_pool(name="outsb", bufs=1) as spool,
            tc.tile_pool(name="psxt", bufs=2, space="PSUM") as ppxt,
            tc.tile_pool(name="pssn", bufs=2, space="PSUM") as ppsn,
            tc.tile_pool(name="psvo", bufs=2, space="PSUM") as ppvo,
        ):
            identb = cpool.tile([128, 128], mm_dtype)
            make_identity(nc, identb[:])
            identf = cpool.tile([128, 128], F32)
            make_identity(nc, identf[:])
            pm_sb = cpool.tile([128, H_BLK * JT], mm_dtype)
            nc.sync.dma_start(out=pm_sb[:], in_=pm_d[:])
            coef_sb = cpool.tile([128, 18], F32)
            nc.sync.dma_start(out=coef_sb[:], in_=coef_d[:])

            S = spool.tile([n_grp * TILES_GRP, 128 * T_IL], F32)  # [(g k), (p t)]
            Sv = S[:].rearrange("q (p t) -> q p t", p=128, t=T_IL)

            KH = TILES_GRP // 2   # tiles per poly half

            def poly_half(sn_ps, vo2, half):
                # sn_ps [128, (k j t)] f32 (PSUM); s_j(row) at [p, k, j, t]
                sv = sn_ps[:].rearrange(
                    "p (k j t) -> p k j t", k=KH, j=4, t=T_IL
                )
                s = [sv[:, :, jj, :] for jj in range(4)]

                def tmp(tag):
                    tl = tpool.tile([128, KH * T_IL], F32, tag=tag)
                    return tl, tl[:].rearrange(
                        "p (k t) -> p k t", k=KH, t=T_IL
                    )

                Lf = []
                for i in range(8):
                    _, v = tmp(f"leaf{i}")
                    nc.scalar.activation(
                        v, s[0], AF.Identity,
                        bias=coef_sb[:, i : i + 1],
                        scale=coef_sb[:, 8 + i : 9 + i],
                    )
                    Lf.append(v)
                G = []
                for k in range(4):
                    _, v = tmp(f"gm{k}")
                    nc.vector.tensor_mul(v, s[1], Lf[2 * k + 1])
                    nc.vector.tensor_add(v, v, Lf[2 * k])
                    G.append(v)
                H = []
                for m in range(2):
                    _, v = tmp(f"hm{m}")
                    nc.vector.tensor_mul(v, s[2], G[2 * m + 1])
                    nc.vector.tensor_add(v, v, G[2 * m])
                    H.append(v)
                _, vl = tmp("lut")
                nc.vector.tensor_mul(vl, s[3], H[1])
                nc.vector.tensor_add(vl, vl, H[0])
                _, va0 = tmp("adds0")
                nc.vector.tensor_copy(out=va0, in_=s[0])
                _, va = tmp("adds")
                nc.vector.tensor_add(va, va0, s[1])
                nc.vector.tensor_add(va, va, s[2])
                _, vr = tmp("relu")
                nc.vector.tensor_scalar(vr, va, -2.0, 0.0, op0=ALU.add, op1=ALU.max)
                nc.vector.tensor_scalar(
                    vr, vr, 1.0, coef_sb[:, 16:17], op0=ALU.min, op1=ALU.mult
                )
                _, vq = tmp("step")
                nc.vector.tensor_scalar(
                    vq, va, 2.0, coef_sb[:, 17:18], op0=ALU.is_ge, op1=ALU.mult
                )
                # write into this half's k-slice of the group vo2 [128, (t k)]
                vo2v = vo2[:].rearrange("p (t k) -> p k t", t=T_IL, k=TILES_GRP)[
                    :, KH * half : KH * (half + 1), :
                ]
                nc.vector.tensor_add(vo2v, vl, vr)
                nc.vector.tensor_add(vo2v, vo2v, vq)

            def store_group(vo2, g):
                # transposes: block b -> voT[:, 128b:...] = [(tl k), p], t = TB*b+tl
                n_blk = TILES_GRP * T_IL // 128
                voT = ppvo.tile([128, 128 * n_blk], F32, tag="voT")
                for b in range(n_blk):
                    nc.tensor.transpose(
                        voT[:, 128 * b : 128 * (b + 1)],
                        vo2[:, 128 * b : 128 * (b + 1)],
                        identf[:],
                    )
                for t in range(T_IL):
                    b, tl = divmod(t, TB)
                    dst = Sv[TILES_GRP * g : TILES_GRP * (g + 1), :, t]
                    srcv = voT[
                        TILES_GRP * tl : TILES_GRP * (tl + 1),
                        128 * b : 128 * (b + 1),
                    ]
                    if t % 2 == 0:
                        nc.scalar.copy(out=dst, in_=srcv)
                    else:
                        nc.vector.tensor_copy(out=dst, in_=srcv)
                nc.sync.dma_start(
                    out=outv[TILES_GRP * g : TILES_GRP * (g + 1)],
                    in_=S[TILES_GRP * g : TILES_GRP * (g + 1), :],
                )

            for g in range(n_grp):
                vo2 = tpool.tile([128, TILES_GRP * T_IL], F32, tag="vo2")
                for half in range(2):
                    sn_ps = ppsn.tile([128, JT * (TILES_GRP // 2)], F32, tag="sn")
                    for kk in range(TILES_GRP // 2):
                        k = (TILES_GRP // 2) * half + kk
                        nd = g * TILES_GRP + k
                        xt = xpool.tile([128, 128 * H_BLK], mm_dtype, tag="x")
                        nc.gpsimd.dma_start(out=xt[:], in_=x2v[nd])
                        xT_ps = ppxt.tile([128, 128 * H_BLK], mm_dtype, tag="xT")
                        for h in range(H_BLK):
                            nc.tensor.transpose(
                                xT_ps[:, 128 * h : 128 * (h + 1)],
                                xt[:, 128 * h : 128 * (h + 1)],
                                identb[:],
                            )
                        xT_sb = xsbpool.tile([128, 128 * H_BLK], mm_dtype, tag="xTsb")
                        if nd % 2 == 0:
                            nc.scalar.copy(out=xT_sb[:], in_=xT_ps[:])
                        else:
                            nc.vector.tensor_copy(out=xT_sb[:], in_=xT_ps[:])
                        for h in range(H_BLK):
                            nc.tensor.matmul(
                                sn_ps[:, JT * kk : JT * (kk + 1)],
                                lhsT=xT_sb[:, 128 * h : 128 * (h + 1)],
                                rhs=pm_sb[:, JT * h : JT * (h + 1)],
                                start=(h == 0),
                                stop=(h == H_BLK - 1),
                            )
                    poly_half(sn_ps, vo2, half)
                store_group(vo2, g)
    nc.compile()
    return nc


def host_prep(choice_parameters, lut, lut_vs_add_choice_parameters, mm_np=ml_dtypes.bfloat16):
    cp = np.asarray(choice_parameters, dtype=np.float64)
    e = np.exp(cp - cp.max(axis=0, keepdims=True))
    probs = e / e.sum(axis=0, keepdims=True)  # [64,4]
    L = 1.0 / (1.0 + np.exp(-np.asarray(lut, dtype=np.float64)))
    m = np.asarray(lut_vs_add_choice_parameters, dtype=np.float64)
    em = np.exp(m - m.max())
    mix = em / em.sum()

    c = np.zeros(16)
    for Sm in range(16):
        v = L
        for ax in range(4):
            vec = np.array([1.0, -1.0]) if (Sm >> ax) & 1 else np.array([0.0, 1.0])
            v = np.tensordot(v, vec, axes=([0], [0]))
        c[Sm] = float(v) * mix[0]

    coef_row = np.zeros(18)
    for idx in range(8):
        coef_row[idx] = c[idx << 1]
        coef_row[8 + idx] = c[(idx << 1) | 1]
    coef_row[16] = mix[1]
    coef_row[17] = mix[2]
    coef = np.tile(coef_row.astype(np.float32)[None], (128, 1))

    # pm[t2*64+c, JT*h + T_IL*j + t] = probs[c,j] * [t == 2h+t2]
    pm = np.zeros((128, H_BLK * JT), np.float64)
    for h in range(H_BLK):
        for t2 in range(2):
            for cc in range(64):
                for j in range(4):
                    pm[t2 * 64 + cc, JT * h + T_IL * j + (2 * h + t2)] = probs[cc, j]
    pm = pm.astype(mm_np)
    return pm, coef


_NC_CACHE = {}


def _get_nc(R):
    if R not in _NC_CACHE:
        _NC_CACHE[R] = build_nc(R)
    return _NC_CACHE[R]


def run_on_hw(x, choice_parameters, lut, lut_vs_add_choice_parameters, **kw):
    x = np.ascontiguousarray(np.asarray(x, dtype=np.float32))
    R = x.shape[0] // N_CORES
    nc = _get_nc(R)
    pm, coef = host_prep(choice_parameters, lut, lut_vs_add_choice_parameters)
    in_maps = [
        {"x": np.ascontiguousarray(x[i * R : (i + 1) * R]), "pm": pm, "coef": coef}
        for i in range(N_CORES)
    ]
    res = run_bass_kernel_spmd(nc, in_maps, list(range(N_CORES)), **kw)
    out = np.concatenate([r["out"] for r in res.results], axis=0)
    return out, res


def kernel(x, choice_parameters, lut, lut_vs_add_choice_parameters):
    out, _ = run_on_hw(x, choice_parameters, lut, lut_vs_add_choice_parameters)
    return out


# revision 13
# speedup vs baseline: 1.1133x; 1.0007x over previous
# Trainium2 Bass kernel for nn_LAB_42906723287350.
#
#   probs = softmax(choice_parameters, axis=0); s = x @ probs
#   out = mix0*multilinear(sigmoid(lut); s) + mix1*clip(s0+s1+s2-2,0,1)
#         + mix2*(s0+s1+s2>=2)
#
# Data parallel over 8 cores (R=131072 rows each).  Row mapping per core:
#   row = ROWS_TILE*nd + T_IL*p + t   (nd = tile, p = partition, t in [0,T_IL))
# Per tile nd: SWDGE cast-DMA x [128, (t,c)] f32->bf16 (4KB/partition reads)
#   -> T_IL/2 PE transposes (bf16) -> xT [(t2,c), p] psum bf16 -> copy
#   -> T_IL/2 accumulating matmuls with xT as STATIONARY and P [128, 4*T_IL]
#      moving -> s in NATURAL layout [128 p, (j,t)] (no back-transpose).
# Per group g (TILES_GRP tiles): copy s psum->SBUF, 16-coeff Horner poly +
#   add path -> vo2 [128, (t,k)] -> PE transposes -> [(tl,k), p] psum
#   -> strided copies into S [(g,k), (p,t)] -> contiguous output DMA
#   (T_IL*512B/partition descriptors).
import numpy as np
import ml_dtypes

import concourse.bass as bass
import concourse.mybir as mybir
import concourse.tile as tile
from concourse import bacc
from concourse.bass_utils import run_bass_kernel_spmd
from concourse.masks import make_identity

N_CORES = 8
B_FULL = 1048576
CIN = 64
F32 = mybir.dt.float32
MM_DT = mybir.dt.bfloat16
ALU = mybir.AluOpType
AF = mybir.ActivationFunctionType

T_IL = 32                        # rows interleaved per partition
H_BLK = T_IL // 2                # 128-partition transpose sub-blocks per tile
XH = min(H_BLK, 8)               # sub-blocks per xT PSUM tile (1 bank)
ROWS_TILE = 128 * T_IL           # 4096 rows per x-tile
TILES_GRP = 32                   # tiles per poly group
ROWS_GRP = ROWS_TILE * TILES_GRP # 131072 rows
JT = 4 * T_IL                    # s-matmul output cols per tile (j,t)
KP = max(1, 1024 // JT)          # tiles per poly part (2-bank PSUM)
TB = 128 // TILES_GRP            # t values per vo transpose block


def build_nc(R, mm_dtype=MM_DT):
    n_tiles = R // ROWS_TILE
    n_grp = n_tiles // TILES_GRP
    assert R == n_grp * ROWS_GRP and n_grp * TILES_GRP <= 128

    nc = bacc.Bacc()
    x_d = nc.dram_tensor("x", [R, CIN], F32, kind="ExternalInput")
    pm_d = nc.dram_tensor("pm", [128, H_BLK * JT], mm_dtype, kind="ExternalInput")
    coef_d = nc.dram_tensor("coef", [128, 18], F32, kind="ExternalInput")
    out_d = nc.dram_tensor("out", [R, 1], F32, kind="ExternalOutput")

    # x row = ROWS_TILE*nd + T_IL*p + t ; tile view [nd, p, (t c)]
    x2v = x_d[:].rearrange("(nd p t) c -> nd p (t c)", p=128, t=T_IL)
    # out row = ROWS_TILE*(g*TILES_GRP+k) + T_IL*p + t ; [(g k), (p t)]
    outv = out_d[:].rearrange(
        "(gk p t) one -> gk (p t one)", gk=n_grp * TILES_GRP, p=128, t=T_IL
    )

    with tile.TileContext(nc) as tc:
        with (
            tc.tile_pool(name="const", bufs=1) as cpool,
            tc.tile_pool(name="xin", bufs=8) as xpool,
            tc.tile_pool(name="xtsb", bufs=6) as xsbpool,
            tc.tile_pool(name="tmp", bufs=2) as tpool,
            tc.tile_pool(name="outsb", bufs=1) as spool,
            tc.tile_pool(name="psxt", bufs=2, space="PSUM") as ppxt,
            tc.tile_pool(name="pssn", bufs=2, space="PSUM") as ppsn,
            tc.tile_pool(name="psvo", bufs=2, space="PSUM") as ppvo,
        ):
            identb = cpool.tile([128, 128], mm_dtype)
            make_identity(nc, identb[:])
            identf = cpool.tile([128, 128], F32)
            make_identity(nc, identf[:])
            pm_sb = cpool.tile([128, H_BLK * JT], mm_dtype)
            nc.sync.dma_start(out=pm_sb[:], in_=pm_d[:])
            coef_sb = cpool.tile([128, 18], F32)
            nc.sync.dma_start(out=coef_sb[:], in_=coef_d[:])

            S = spool.tile([n_grp * TILES_GRP, 128 * T_IL], F32)  # [(g k), (p t)]
            Sv = S[:].rearrange("q (p t) -> q p t", p=128, t=T_IL)

            def poly_part(sn_ps, vo2, part, last=False):
                # sn_ps [128, (k j t)] f32 (PSUM); s_j(row) at [p, k, j, t]
                sv = sn_ps[:].rearrange(
                    "p (k j t) -> p k j t", k=KP, j=4, t=T_IL
                )
                s = [sv[:, :, jj, :] for jj in range(4)]

                def tmp(tag):
                    tl = tpool.tile([128, KP * T_IL], F32, tag=tag)
                    return tl, tl[:].rearrange(
                        "p (k t) -> p k t", k=KP, t=T_IL
                    )

                Lf = []
                for i in range(8):
                    _, v = tmp(f"leaf{i}")
                    if last:
                        nc.vector.tensor_scalar(
                            v, s[0], coef_sb[:, 8 + i : 9 + i],
                            coef_sb[:, i : i + 1], op0=ALU.mult, op1=ALU.add,
                        )
                    else:
                        nc.scalar.activation(
                            v, s[0], AF.Identity,
                            bias=coef_sb[:, i : i + 1],
                            scale=coef_sb[:, 8 + i : 9 + i],
                        )
                    Lf.append(v)
                G = []
                for k in range(4):
                    _, v = tmp(f"gm{k}")
                    nc.vector.tensor_mul(v, s[1], Lf[2 * k + 1])
                    nc.vector.tensor_add(v, v, Lf[2 * k])
                    G.append(v)
                H = []
                for m in range(2):
                    _, v = tmp(f"hm{m}")
                    nc.vector.tensor_mul(v, s[2], G[2 * m + 1])
                    nc.vector.tensor_add(v, v, G[2 * m])
                    H.append(v)
                _, vl = tmp("lut")
                nc.vector.tensor_mul(vl, s[3], H[1])
                nc.vector.tensor_add(vl, vl, H[0])
                _, va0 = tmp("adds0")
                nc.vector.tensor_copy(out=va0, in_=s[0])
                _, va = tmp("adds")
                nc.vector.tensor_add(va, va0, s[1])
                nc.vector.tensor_add(va, va, s[2])
                _, vr = tmp("relu")
                nc.vector.tensor_scalar(vr, va, -2.0, 0.0, op0=ALU.add, op1=ALU.max)
                nc.vector.tensor_scalar(
                    vr, vr, 1.0, coef_sb[:, 16:17], op0=ALU.min, op1=ALU.mult
                )
                _, vq = tmp("step")
                nc.vector.tensor_scalar(
                    vq, va, 2.0, coef_sb[:, 17:18], op0=ALU.is_ge, op1=ALU.mult
                )
                # write into this part's k-slice of the group vo2 [128, (t k)]
                vo2v = vo2[:].rearrange("p (t k) -> p k t", t=T_IL, k=TILES_GRP)[
                    :, KP * part : KP * (part + 1), :
                ]
                nc.vector.tensor_add(vo2v, vl, vr)
                nc.vector.tensor_add(vo2v, vo2v, vq)

            def store_group(vo2, g):
                # transposes: block b -> voT[:, ...] = [(tl k), p], t = TB*b+tl
                n_blk = TILES_GRP * T_IL // 128
                # split into PSUM-bank-sized chunks of 4 blocks
                for vh in range(n_blk // 4):
                    voT = ppvo.tile([128, 512], F32, tag="voT")
                    for bb in range(4):
                        b = 4 * vh + bb
                        nc.tensor.transpose(
                            voT[:, 128 * bb : 128 * (bb + 1)],
                            vo2[:, 128 * b : 128 * (b + 1)],
                            identf[:],
                        )
                    voTv = voT[:].rearrange("q (b p) -> q p b", b=4, p=128)
                    for tl in range(TB):
                        # blocks bb=0..3 of this chunk hit t = 16*vh + 4*bb + tl
                        t0 = 4 * TB * vh + tl
                        dst = Sv[
                            TILES_GRP * g : TILES_GRP * (g + 1), :,
                            t0 : t0 + 3 * TB + 1 : TB,
                        ]
                        srcv = voTv[TILES_GRP * tl : TILES_GRP * (tl + 1)]
                        if tl % 2 == 0:
                            nc.scalar.copy(out=dst, in_=srcv)
                        else:
                            nc.vector.tensor_copy(out=dst, in_=srcv)
                nc.sync.dma_start(
                    out=outv[TILES_GRP * g : TILES_GRP * (g + 1)],
                    in_=S[TILES_GRP * g : TILES_GRP * (g + 1), :],
                )

            for g in range(n_grp):
                vo2 = tpool.tile([128, TILES_GRP * T_IL], F32, tag="vo2")
                for part in range(TILES_GRP // KP):
                    sn_ps = ppsn.tile([128, JT * KP], F32, tag="sn")
                    for kk in range(KP):
                        k = KP * part + kk
                        nd = g * TILES_GRP + k
                        xt = xpool.tile([128, 128 * H_BLK], mm_dtype, tag="x")
                        nc.gpsimd.dma_start(out=xt[:], in_=x2v[nd])
                        xT_sb = xsbpool.tile([128, 128 * H_BLK], mm_dtype, tag="xTsb")
                        for hh in range(H_BLK // XH):
                            xT_ps = ppxt.tile([128, 128 * XH], mm_dtype, tag="xT")
                            for h2 in range(XH):
                                h = XH * hh + h2
                                nc.tensor.transpose(
                                    xT_ps[:, 128 * h2 : 128 * (h2 + 1)],
                                    xt[:, 128 * h : 128 * (h + 1)],
                                    identb[:],
                                )
                            dst = xT_sb[:, 128 * XH * hh : 128 * XH * (hh + 1)]
                            if (nd + hh) % 2 == 0:
                                nc.scalar.copy(out=dst, in_=xT_ps[:])
                            else:
                                nc.vector.tensor_copy(out=dst, in_=xT_ps[:])
                        for h in range(H_BLK):
                            nc.tensor.matmul(
                                sn_ps[:, JT * kk : JT * (kk + 1)],
                                lhsT=xT_sb[:, 128 * h : 128 * (h + 1)],
                                rhs=pm_sb[:, JT * h : JT * (h + 1)],
                                start=(h == 0),
                                stop=(h == H_BLK - 1),
                            )
                    poly_part(
                        sn_ps, vo2, part, last=(part == TILES_GRP // KP - 1)
                    )
                store_group(vo2, g)
    nc.compile()
    return nc


def host_prep(choice_parameters, lut, lut_vs_add_choice_parameters, mm_np=ml_dtypes.bfloat16):
    cp = np.asarray(choice_parameters, dtype=np.float64)
    e = np.exp(cp - cp.max(axis=0, keepdims=True))
    probs = e / e.sum(axis=0, keepdims=True)  # [64,4]
    L = 1.0 / (1.0 + np.exp(-np.asarray(lut, dtype=np.float64)))
    m = np.asarray(lut_vs_add_choice_parameters, dtype=np.float64)
    em = np.exp(m - m.max())
    mix = em / em.sum()

    c = np.zeros(16)
    for Sm in range(16):
        v = L
        for ax in range(4):
            vec = np.array([1.0, -1.0]) if (Sm >> ax) & 1 else np.array([0.0, 1.0])
            v = np.tensordot(v, vec, axes=([0], [0]))
        c[Sm] = float(v) * mix[0]

    coef_row = np.zeros(18)
    for idx in range(8):
        coef_row[idx] = c[idx << 1]
        coef_row[8 + idx] = c[(idx << 1) | 1]
    coef_row[16] = mix[1]
    coef_row[17] = mix[2]
    coef = np.tile(coef_row.astype(np.float32)[None], (128, 1))

    # pm[t2*64+c, JT*h + T_IL*j + t] = probs[c,j] * [t == 2h+t2]
    pm = np.zeros((128, H_BLK * JT), np.float64)
    for h in range(H_BLK):
        for t2 in range(2):
            for cc in range(64):
                for j in range(4):
                    pm[t2 * 64 + cc, JT * h + T_IL * j + (2 * h + t2)] = probs[cc, j]
    pm = pm.astype(mm_np)
    return pm, coef


_NC_CACHE = {}


def _get_nc(R):
    if R not in _NC_CACHE:
        _NC_CACHE[R] = build_nc(R)
    return _NC_CACHE[R]


def run_on_hw(x, choice_parameters, lut, lut_vs_add_choice_parameters, **kw):
    x = np.ascontiguousarray(np.asarray(x, dtype=np.float32))
    R = x.shape[0] // N_CORES
    nc = _get_nc(R)
    pm, coef = host_prep(choice_parameters, lut, lut_vs_add_choice_parameters)
    in_maps = [
        {"x": np.ascontiguousarray(x[i * R : (i + 1) * R]), "pm": pm, "coef": coef}
        for i in range(N_CORES)
    ]
    res = run_bass_kernel_spmd(nc, in_maps, list(range(N_CORES)), **kw)
    out = np.concatenate([r["out"] for r in res.results], axis=0)
    return out, res


def kernel(x, choice_parameters, lut, lut_vs_add_choice_parameters):
    out, _ = run_on_hw(x, choice_parameters, lut, lut_vs_add_choice_parameters)
    return out


# revision 21
# speedup vs baseline: 1.1163x; 1.0027x over previous
# Trainium2 Bass kernel for nn_LAB_42906723287350.
#
#   probs = softmax(choice_parameters, axis=0); s = x @ probs
#   out = mix0*multilinear(sigmoid(lut); s) + mix1*clip(s0+s1+s2-2,0,1)
#         + mix2*(s0+s1+s2>=2)
#
# Data parallel over 8 cores (R=131072 rows each).  Row mapping per core:
#   row = ROWS_TILE*nd + T_IL*p + t   (nd = tile, p = partition, t in [0,T_IL))
# Per tile nd: SWDGE cast-DMA x [128, (t,c)] f32->bf16 (4KB/partition reads)
#   -> T_IL/2 PE transposes (bf16) -> xT [(t2,c), p] psum bf16 -> copy
#   -> T_IL/2 accumulating matmuls with xT as STATIONARY and P [128, 4*T_IL]
#      moving -> s in NATURAL layout [128 p, (j,t)] (no back-transpose).
# Per group g (TILES_GRP tiles): copy s psum->SBUF, 16-coeff Horner poly +
#   add path -> vo2 [128, (t,k)] -> PE transposes -> [(tl,k), p] psum
#   -> strided copies into S [(g,k), (p,t)] -> contiguous output DMA
#   (T_IL*512B/partition descriptors).
import numpy as np
import ml_dtypes

import concourse.bass as bass
import concourse.mybir as mybir
import concourse.tile as tile
from concourse import bacc
from concourse.bass_utils import run_bass_kernel_spmd
from concourse.masks import make_identity

N_CORES = 8
B_FULL = 1048576
CIN = 64
F32 = mybir.dt.float32
MM_DT = mybir.dt.bfloat16
ALU = mybir.AluOpType
AF = mybir.ActivationFunctionType

T_IL = 32                        # rows interleaved per partition
H_BLK = T_IL // 2                # 128-partition transpose sub-blocks per tile
XH = min(H_BLK, 8)               # sub-blocks per xT PSUM tile (1 bank)
ROWS_TILE = 128 * T_IL           # 4096 rows per x-tile
TILES_GRP = 32                   # tiles per poly group
ROWS_GRP = ROWS_TILE * TILES_GRP # 131072 rows
JT = 4 * T_IL                    # s-matmul output cols per tile (j,t)
KP = max(1, 1024 // JT)          # tiles per poly part (2-bank PSUM)
TB = 128 // TILES_GRP            # t values per vo transpose block


def build_nc(R, mm_dtype=MM_DT):
    n_tiles = R // ROWS_TILE
    n_grp = n_tiles // TILES_GRP
    assert R == n_grp * ROWS_GRP and n_grp * TILES_GRP <= 128

    nc = bacc.Bacc()
    x_d = nc.dram_tensor("x", [R, CIN], F32, kind="ExternalInput")
    pm_d = nc.dram_tensor("pm", [128, H_BLK * JT], mm_dtype, kind="ExternalInput")
    coef_d = nc.dram_tensor("coef", [128, 18], F32, kind="ExternalInput")
    out_d = nc.dram_tensor("out", [R, 1], F32, kind="ExternalOutput")

    # x row = ROWS_TILE*nd + T_IL*p + t ; tile view [nd, p, (t c)]
    x2v = x_d[:].rearrange("(nd p t) c -> nd p (t c)", p=128, t=T_IL)
    # out row = ROWS_TILE*(g*TILES_GRP+k) + T_IL*p + t ; [(g k), (p t)]
    outv = out_d[:].rearrange(
        "(gk p t) one -> gk (p t one)", gk=n_grp * TILES_GRP, p=128, t=T_IL
    )

    with tile.TileContext(nc) as tc:
        with (
            tc.tile_pool(name="const", bufs=1) as cpool,
            tc.tile_pool(name="xin", bufs=10) as xpool,
            tc.tile_pool(name="xtsb", bufs=8) as xsbpool,
            tc.tile_pool(name="tmp", bufs=2) as tpool,
            tc.tile_pool(name="outsb", bufs=1) as spool,
            tc.tile_pool(name="psxt", bufs=2, space="PSUM") as ppxt,
            tc.tile_pool(name="pssn", bufs=2, space="PSUM") as ppsn,
            tc.tile_pool(name="psvo", bufs=2, space="PSUM") as ppvo,
        ):
            identb = cpool.tile([128, 128], mm_dtype)
            make_identity(nc, identb[:])
            identf = cpool.tile([128, 128], F32)
            make_identity(nc, identf[:])
            pm_sb = cpool.tile([128, H_BLK * JT], mm_dtype)
            nc.sync.dma_start(out=pm_sb[:], in_=pm_d[:])
            coef_sb = cpool.tile([128, 18], F32)
            nc.sync.dma_start(out=coef_sb[:], in_=coef_d[:])

            S = spool.tile([n_grp * TILES_GRP, 128 * T_IL], F32)  # [(g k), (p t)]
            Sv = S[:].rearrange("q (p t) -> q p t", p=128, t=T_IL)

            def poly_part(sn_ps, vo2, part, last=False):
                # sn_ps [128, (k j t)] f32 (PSUM); s_j(row) at [p, k, j, t]
                sv = sn_ps[:].rearrange(
                    "p (k j t) -> p k j t", k=KP, j=4, t=T_IL
                )
                s = [sv[:, :, jj, :] for jj in range(4)]

                def tmp(tag):
                    tl = tpool.tile([128, KP * T_IL], F32, tag=tag)
                    return tl, tl[:].rearrange(
                        "p (k t) -> p k t", k=KP, t=T_IL
                    )

                Lf = []
                for i in range(8):
                    _, v = tmp(f"leaf{i}")
                    if last:
                        nc.vector.tensor_scalar(
                            v, s[0], coef_sb[:, 8 + i : 9 + i],
                            coef_sb[:, i : i + 1], op0=ALU.mult, op1=ALU.add,
                        )
                    else:
                        nc.scalar.activation(
                            v, s[0], AF.Identity,
                            bias=coef_sb[:, i : i + 1],
                            scale=coef_sb[:, 8 + i : 9 + i],
                        )
                    Lf.append(v)
                G = []
                for k in range(4):
                    _, v = tmp(f"gm{k}")
                    nc.vector.tensor_mul(v, s[1], Lf[2 * k + 1])
                    nc.vector.tensor_add(v, v, Lf[2 * k])
                    G.append(v)
                H = []
                for m in range(2):
                    _, v = tmp(f"hm{m}")
                    nc.vector.tensor_mul(v, s[2], G[2 * m + 1])
                    nc.vector.tensor_add(v, v, G[2 * m])
                    H.append(v)
                _, vl = tmp("lut")
                nc.vector.tensor_mul(vl, s[3], H[1])
                nc.vector.tensor_add(vl, vl, H[0])
                _, va0 = tmp("adds0")
                nc.vector.tensor_copy(out=va0, in_=s[0])
                _, va = tmp("adds")
                nc.vector.tensor_add(va, va0, s[1])
                nc.vector.tensor_add(va, va, s[2])
                _, vr = tmp("relu")
                nc.vector.tensor_scalar(vr, va, -2.0, 0.0, op0=ALU.add, op1=ALU.max)
                nc.vector.tensor_scalar(
                    vr, vr, 1.0, coef_sb[:, 16:17], op0=ALU.min, op1=ALU.mult
                )
                _, vq = tmp("step")
                nc.vector.tensor_scalar(
                    vq, va, 2.0, coef_sb[:, 17:18], op0=ALU.is_ge, op1=ALU.mult
                )
                # write into this part's k-slice of the group vo2 [128, (t k)]
                vo2v = vo2[:].rearrange("p (t k) -> p k t", t=T_IL, k=TILES_GRP)[
                    :, KP * part : KP * (part + 1), :
                ]
                nc.vector.tensor_add(vo2v, vl, vr)
                nc.vector.tensor_add(vo2v, vo2v, vq)

            def store_group(vo2, g):
                # transposes: block b -> voT[:, ...] = [(tl k), p], t = TB*b+tl
                n_blk = TILES_GRP * T_IL // 128
                # split into PSUM-bank-sized chunks of 4 blocks
                for vh in range(n_blk // 4):
                    voT = ppvo.tile([128, 512], F32, tag="voT")
                    for bb in range(4):
                        b = 4 * vh + bb
                        nc.tensor.transpose(
                            voT[:, 128 * bb : 128 * (bb + 1)],
                            vo2[:, 128 * b : 128 * (b + 1)],
                            identf[:],
                        )
                    voTv = voT[:].rearrange("q (b p) -> q p b", b=4, p=128)
                    for tl in range(TB):
                        # blocks bb=0..3 of this chunk hit t = 16*vh + 4*bb + tl
                        t0 = 4 * TB * vh + tl
                        dst = Sv[
                            TILES_GRP * g : TILES_GRP * (g + 1), :,
                            t0 : t0 + 3 * TB + 1 : TB,
                        ]
                        srcv = voTv[TILES_GRP * tl : TILES_GRP * (tl + 1)]
                        if tl % 2 == 0:
                            nc.scalar.copy(out=dst, in_=srcv)
                        else:
                            nc.vector.tensor_copy(out=dst, in_=srcv)
                nc.sync.dma_start(
                    out=outv[TILES_GRP * g : TILES_GRP * (g + 1)],
                    in_=S[TILES_GRP * g : TILES_GRP * (g + 1), :],
                )

            for g in range(n_grp):
                vo2 = tpool.tile([128, TILES_GRP * T_IL], F32, tag="vo2")
                for part in range(TILES_GRP // KP):
                    sn_ps = ppsn.tile([128, JT * KP], F32, tag="sn")
                    for kk in range(KP):
                        k = KP * part + kk
                        nd = g * TILES_GRP + k
                        xt = xpool.tile([128, 128 * H_BLK], mm_dtype, tag="x")
                        nc.gpsimd.dma_start(out=xt[:], in_=x2v[nd])
                        xT_sb = xsbpool.tile([128, 128 * H_BLK], mm_dtype, tag="xTsb")
                        for hh in range(H_BLK // XH):
                            xT_ps = ppxt.tile([128, 128 * XH], mm_dtype, tag="xT")
                            for h2 in range(XH):
                                h = XH * hh + h2
                                nc.tensor.transpose(
                                    xT_ps[:, 128 * h2 : 128 * (h2 + 1)],
                                    xt[:, 128 * h : 128 * (h + 1)],
                                    identb[:],
                                )
                            dst = xT_sb[:, 128 * XH * hh : 128 * XH * (hh + 1)]
                            if (nd + hh) % 2 == 0:
                                nc.scalar.copy(out=dst, in_=xT_ps[:])
                            else:
                                nc.vector.tensor_copy(out=dst, in_=xT_ps[:])
                        for h in range(H_BLK):
                            nc.tensor.matmul(
                                sn_ps[:, JT * kk : JT * (kk + 1)],
                                lhsT=xT_sb[:, 128 * h : 128 * (h + 1)],
                                rhs=pm_sb[:, JT * h : JT * (h + 1)],
                                start=(h == 0),
                                stop=(h == H_BLK - 1),
                            )
                    poly_part(
                        sn_ps, vo2, part, last=(part == TILES_GRP // KP - 1)
                    )
                store_group(vo2, g)
    nc.compile()
    return nc


def host_prep(choice_parameters, lut, lut_vs_add_choice_parameters, mm_np=ml_dtypes.bfloat16):
    cp = np.asarray(choice_parameters, dtype=np.float64)
    e = np.exp(cp - cp.max(axis=0, keepdims=True))
    probs = e / e.sum(axis=0, keepdims=True)  # [64,4]
    L = 1.0 / (1.0 + np.exp(-np.asarray(lut, dtype=np.float64)))
    m = np.asarray(lut_vs_add_choice_parameters, dtype=np.float64)
    em = np.exp(m - m.max())
    mix = em / em.sum()

    c = np.zeros(16)
    for Sm in range(16):
        v = L
        for ax in range(4):
            vec = np.array([1.0, -1.0]) if (Sm >> ax) & 1 else np.array([0.0, 1.0])
            v = np.tensordot(v, vec, axes=([0], [0]))
        c[Sm] = float(v) * mix[0]

    coef_row = np.zeros(18)
    for idx in range(8):
        coef_row[idx] = c[idx << 1]
        coef_row[8 + idx] = c[(idx << 1) | 1]
    coef_row[16] = mix[1]
    coef_row[17] = mix[2]
    coef = np.tile(coef_row.astype(np.float32)[None], (128, 1))

    # pm[t2*64+c, JT*h + T_IL*j + t] = probs[c,j] * [t == 2h+t2]
    pm = np.zeros((128, H_BLK * JT), np.float64)
    for h in range(H_BLK):
        for t2 in range(2):
            for cc in range(64):
                for j in range(4):
                    pm[t2 * 64 + cc, JT * h + T_IL * j + (2 * h + t2)] = probs[cc, j]
    pm = pm.astype(mm_np)
    return pm, coef


_NC_CACHE = {}


def _get_nc(R):
    if R not in _NC_CACHE:
        _NC_CACHE[R] = build_nc(R)
    return _NC_CACHE[R]


def run_on_hw(x, choice_parameters, lut, lut_vs_add_choice_parameters, **kw):
    x = np.ascontiguousarray(np.asarray(x, dtype=np.float32))
    R = x.shape[0] // N_CORES
    nc = _get_nc(R)
    pm, coef = host_prep(choice_parameters, lut, lut_vs_add_choice_parameters)
    in_maps = [
        {"x": np.ascontiguousarray(x[i * R : (i + 1) * R]), "pm": pm, "coef": coef}
        for i in range(N_CORES)
    ]
    res = run_bass_kernel_spmd(nc, in_maps, list(range(N_CORES)), **kw)
    out = np.concatenate([r["out"] for r in res.results], axis=0)
    return out, res


def kernel(x, choice_parameters, lut, lut_vs_add_choice_parameters):
    out, _ = run_on_hw(x, choice_parameters, lut, lut_vs_add_choice_parameters)
    return out
